# revision 36
# baseline (speedup 1.0000x reference)
"""Trainium2 kernel for joint NER (linear + CRF Viterbi) + biaffine relation scoring.

Strategy:
  - Host (numpy): tiny NER linear [4,256,768]@[768,9], sequential Viterbi DP
    (255 steps over a [4,9] state -- latency-bound, ~0.1% of FLOPs), label
    embedding gather, final diagonal set.
  - Device (8 NeuronCores, Bass/Tile): the heavy biaffine part.
    Shard over (batch b, target-half ih): core c -> b=c//2, i in [ih*128,(ih+1)*128).
    Per core (h padded 798 -> 896 = 7*128 chunks; pad rows hit V=0 -> no effect):
      uT[h,i] = (U_w.T @ hx_b.T)[h, i-slice] + (U_b+W_b)[h]  (bf16 mm, fp32 psum)
      wT[h,j] = (W_w.T @ hx_b.T)[h,j], duplicated 2x -> wsb2[h, j, 2]
      pair[h, ip, j, 2] = tanh(wsb2 + u-broadcast)     (DVE 2x_1p bf16 add, ACT tanh)
      out[i, r, (j,i2)] = sum_h V[h,r] * pair          (PE bf16, psum fp32)
    The 2-wide i2 inner axis keeps every DVE operand packed (step-1 innermost,
    2-byte -> 2x_1p) while the matmul moving operand stays contiguous. The
    mostly-pad 7th h-chunk is packed 4x across partitions so its tanh costs
    512 free cycles instead of 2048, then DMA-unpacked for the matmul.
    Pipeline: 17 i-blocks, pair bufs=4; warm-up matmuls ramp the PE clock
    during the weight DMAs; block-0 adds stream inline behind phase B.
"""

import sys
import os
import numpy as np

sys.path.insert(0, "/opt/trn_rl_repo")

B, S, D, T, L, R = 4, 256, 768, 9, 30, 12
H = D + L          # 798
HP = 896           # 7 * 128, padded h
NCH = 7            # h chunks of 128
IB = 16            # i-values per block
NBLK = 128 // IB   # 8 blocks per core
NCORES = 8

_CACHE = {}


def _viterbi_np(emissions, start_trans, trans, end_trans):
    """Batched Viterbi decode, numpy, matches the jax reference step-for-step."""
    b, s, t = emissions.shape
    score = start_trans[None, :] + emissions[:, 0]            # [B,T]
    hist = np.zeros((s - 1, b, t), np.int64)
    for step in range(1, s):
        nxt = score[:, :, None] + trans[None, :, :]           # [B,Tprev,Tnext]
        hist[step - 1] = np.argmax(nxt, axis=1)
        score = np.max(nxt, axis=1) + emissions[:, step]
    last = np.argmax(score + end_trans[None, :], axis=1)      # [B]
    tags = np.zeros((b, s), np.int32)
    tags[:, -1] = last
    cur = last
    ar = np.arange(b)
    for step in range(s - 2, -1, -1):
        cur = hist[step][ar, cur]
        tags[:, step] = cur
    return tags


def _build():
    """Build + compile the SPMD Bass program (one program, 8 data shards)."""
    from concourse import bass, mybir, tile, bacc

    f32 = mybir.dt.float32
    bf16 = mybir.dt.bfloat16
    ADD = mybir.AluOpType.add
    Tanh = mybir.ActivationFunctionType.Tanh

    nc = bacc.Bacc("TRN2", target_bir_lowering=False, debug=False, num_devices=NCORES)

    hxT_d = nc.dram_tensor("hxT", (128, NCH, 256), bf16, kind="ExternalInput").ap()
    hxiT_d = nc.dram_tensor("hxiT", (128, NCH, 128), bf16, kind="ExternalInput").ap()
    uw_d = nc.dram_tensor("uw", (128, NCH, HP), bf16, kind="ExternalInput").ap()
    ww_d = nc.dram_tensor("ww", (128, NCH, HP), bf16, kind="ExternalInput").ap()
    vw_d = nc.dram_tensor("vw", (128, NCH, R), bf16, kind="ExternalInput").ap()
    ubwb_d = nc.dram_tensor("ubwb", (128, NCH), f32, kind="ExternalInput").ap()
    out_d = nc.dram_tensor("out", (64, R, 512), f32, kind="ExternalOutput").ap()

    with tile.TileContext(nc) as tc:
        with tc.tile_pool(name="persist", bufs=1) as pers:
            v_sb = pers.tile([128, NCH, R], bf16, tag="v")
            ubwb_sb = pers.tile([128, NCH], f32, tag="ubwb")
            # w replicated 2x along an inner i-pair axis: keeps every DVE
            # operand packed (2x_1p) while the matmul rhs stays contiguous.
            # One tile per chunk so phase-D deps are per-chunk exact.
            wsb2 = [
                pers.tile([128, 256, 2], bf16, tag=f"wsb2_{hc}", name=f"wsb2_{hc}")
                for hc in range(NCH)
            ]
            usb = pers.tile([128, NCH, 128], bf16, tag="usb")

            warm_sb = pers.tile([1, 1], f32, tag="warm")
            nc.sync.dma_start(ubwb_sb[:], ubwb_d[:])
            nc.sync.dma_start(v_sb[:], vw_d[:])
            # preload the Tanh ACT table off the critical path
            nc.scalar.activation(warm_sb[:], ubwb_sb[0:1, 0:1], Tanh)

            with (
                tc.tile_pool(name="wts", bufs=1) as wts,
                tc.tile_pool(name="bpsum", bufs=2, space="PSUM") as bp,
                tc.tile_pool(name="pair", bufs=4) as pp,
                tc.tile_pool(name="ostage", bufs=3) as op_,
                tc.tile_pool(name="spsum", bufs=6, space="PSUM") as sp,
            ):
                hxiT_sb = wts.tile([128, NCH, 128], bf16, tag="hxiT")
                uw_sb = wts.tile([128, NCH, HP], bf16, tag="uw")
                hxT_sb = wts.tile([128, NCH, 256], bf16, tag="hxT")
                ww_sb = wts.tile([128, NCH, HP], bf16, tag="ww")
                nc.sync.dma_start(hxiT_sb[:], hxiT_d[:])
                # uw lands per-chunk so the uT matmuls stream behind the DMA
                for kc in range(NCH):
                    nc.sync.dma_start(uw_sb[:, kc, :], uw_d[:, kc, :])
                nc.sync.dma_start(hxT_sb[:], hxT_d[:])
                nc.sync.dma_start(ww_sb[:], ww_d[:])

                # keep the PE busy while the weight DMAs land so the HAM
                # clock-gate ramps to 2.4GHz before the real matmuls start
                wk = wts.tile([128, 512], bf16, tag="wk")
                nc.gpsimd.memset(wk[:], 0)
                wkp = bp.tile([128, 512], f32, tag="bp", name="warmps")
                for r_ in range(12):
                    nc.tensor.matmul(
                        wkp[:], wk[:, 0:128], wk[:], start=True, stop=True
                    )

                # uT first, kc-outer so each matmul only waits for its own
                # uw chunk DMA; 6 accumulators borrowed from the (still idle)
                # phase-D psum pool + 1 from bp. Fold (U_b + W_b) into usb so
                # the wT copy can be a plain ACT-engine Copy.
                ups = [
                    sp.tile([128, 512], f32, tag="ps", name=f"up{hc}")
                    if hc < 6
                    else bp.tile([128, 128], f32, tag="bp", name=f"up{hc}")
                    for hc in range(NCH)
                ]
                for kc in range(NCH):
                    for hc in range(NCH):
                        out_ap = ups[hc][:, 0:128] if hc < 6 else ups[hc][:]
                        nc.tensor.matmul(
                            out_ap,
                            uw_sb[:, kc, hc * 128 : (hc + 1) * 128],
                            hxiT_sb[:, kc, :],
                            start=(kc == 0),
                            stop=(kc == NCH - 1),
                        )
                for hc in range(NCH):
                    in_ap = ups[hc][:, 0:128] if hc < 6 else ups[hc][:]
                    nc.vector.tensor_scalar(
                        usb[:, hc, :], in_ap, ubwb_sb[:, hc : hc + 1], None, ADD
                    )
                pt0 = pp.tile([128, NCH, 4, 256, 2], bf16, tag="pair", name="pt0")
                for hc in range(NCH):
                    wp = bp.tile([128, 256], f32, tag="bp", name=f"wp{hc}")
                    for kc in range(NCH):
                        nc.tensor.matmul(
                            wp[:],
                            ww_sb[:, kc, hc * 128 : (hc + 1) * 128],
                            hxT_sb[:, kc, :],
                            start=(kc == 0),
                            stop=(kc == NCH - 1),
                        )
                    # one broadcast-read copy fills both duplicate slots
                    nc.scalar.copy(
                        wsb2[hc][:], wp[:].unsqueeze(2).to_broadcast((128, 256, 2))
                    )
                    if hc < 6:
                        # block 0's add immediately behind each chunk, so the
                        # first tanh isn't gated on the whole wT stream
                        u_v0 = (
                            usb[:, hc, 0:8]
                            .rearrange("p (a b) -> p a b", a=4)
                            .unsqueeze(2)
                            .to_broadcast((128, 4, 256, 2))
                        )
                        w_b0 = (
                            wsb2[hc][:]
                            .unsqueeze(1)
                            .to_broadcast((128, 4, 256, 2))
                        )
                        nc.vector.tensor_tensor(pt0[:, hc], w_b0, u_v0, ADD)

                # chunk 6 holds only 30 real h rows; pack its w/u 4x along
                # partitions so its tanh costs 512 free cycles instead of 2048
                u6p = pers.tile([120, 128], bf16, tag="u6p")
                w6p = pers.tile([120, 64, 2], bf16, tag="w6p")
                for q in range(4):
                    nc.sync.dma_start(u6p[q * 30 : (q + 1) * 30, :], usb[0:30, 6, :])
                    nc.sync.dma_start(
                        w6p[q * 30 : (q + 1) * 30],
                        wsb2[6][0:30, q * 64 : (q + 1) * 64, :],
                    )

                # 15 blocks of 8 i, then 2 of 4 i (smaller tail after the
                # last tanh)
                blocks = [(hb * 8, 8) for hb in range(15)] + [(120, 4), (124, 4)]
                pending = []  # (pss, i0, a) whose psum->sbuf drain is deferred
                for i0, cnt in blocks:
                    a = cnt // 2  # i-pairs in this block
                    # pair layout [h, (hc, ipair, j, i2)]: i2 innermost keeps
                    # DVE packed; (j, i2) is contiguous 512 for the matmul rhs
                    if i0 == 0:
                        pt = pt0  # adds already emitted inline with phase B
                    else:
                        pt = pp.tile([128, NCH, a, 256, 2], bf16, tag="pair")
                        for hc in range(6):
                            u_v = (
                                usb[:, hc, i0 : i0 + cnt]
                                .rearrange("p (a b) -> p a b", a=a)
                                .unsqueeze(2)
                                .to_broadcast((128, a, 256, 2))
                            )
                            w_b = (
                                wsb2[hc][:]
                                .unsqueeze(1)
                                .to_broadcast((128, a, 256, 2))
                            )
                            nc.vector.tensor_tensor(pt[:, hc], w_b, u_v, ADD)
                    # chunk 6 in the packed [120, ...] layout
                    p6 = pp.tile([120, a, 64, 2], bf16, tag="p6")
                    u6v = (
                        u6p[:, i0 : i0 + cnt]
                        .rearrange("p (a b) -> p a b", a=a)
                        .unsqueeze(2)
                        .to_broadcast((120, a, 64, 2))
                    )
                    w6v = w6p[:].unsqueeze(1).to_broadcast((120, a, 64, 2))
                    nc.vector.tensor_tensor(p6[:], w6v, u6v, ADD)
                    nc.scalar.activation(
                        p6[:].rearrange("p a c d -> p (a c d)"),
                        p6[:].rearrange("p a c d -> p (a c d)"),
                        Tanh,
                    )
                    # unpack the 4 j-quarters back to [30, ...] for the matmul
                    for q in range(4):
                        nc.sync.dma_start(
                            pt[0:30, 6, :, q * 64 : (q + 1) * 64, :],
                            p6[q * 30 : (q + 1) * 30],
                        )
                    nc.scalar.activation(
                        pt[:, 0:6].rearrange("p a b c d -> p (a b c d)"),
                        pt[:, 0:6].rearrange("p a b c d -> p (a b c d)"),
                        Tanh,
                    )

                    # hc-outer: 4 psum tiles alive, 4 back-to-back matmuls
                    # per stationary V chunk load
                    pss = [
                        sp.tile([128, 512], f32, tag="ps", name=f"ps_{i0}_{t}")
                        for t in range(a)
                    ]
                    for hc in range(6):
                        for t in range(a):
                            nc.tensor.matmul(
                                pss[t][0:R, :],
                                v_sb[:, hc, :],
                                pt[:, hc, t],
                                start=(hc == 0),
                                stop=False,
                            )
                    for t in range(a):
                        # chunk 6: contract only the 30 real h rows
                        nc.tensor.matmul(
                            pss[t][0:R, :],
                            v_sb[0:30, 6, :],
                            pt[0:30, 6, t],
                            start=False,
                            stop=True,
                        )
                    # drain half now, defer half until after the next
                    # block's adds -- keeps DVE feeding ACT first while only
                    # 6 psum banks stay live
                    for t in range(2, a):
                        osb = op_.tile([128, 512], f32, tag="osb")
                        nc.vector.tensor_copy(osb[0:R, :], pss[t][0:R, :])
                        nc.sync.dma_start(out_d[i0 // 2 + t], osb[0:R, :])
                    pending.append((pss, i0, min(a, 2)))
                    if len(pending) > 1:
                        ppss, pi0, pa = pending.pop(0)
                        for t in range(pa):
                            osb = op_.tile([128, 512], f32, tag="osb")
                            nc.vector.tensor_copy(osb[0:R, :], ppss[t][0:R, :])
                            nc.sync.dma_start(out_d[pi0 // 2 + t], osb[0:R, :])
                for ppss, pi0, pa in pending:
                    for t in range(pa):
                        osb = op_.tile([128, 512], f32, tag="osb")
                        nc.vector.tensor_copy(osb[0:R, :], ppss[t][0:R, :])
                        nc.sync.dma_start(out_d[pi0 // 2 + t], osb[0:R, :])

    nc.compile()
    return nc


def _ensure_ntff_hook():
    """Register the axon NTFF profiling hook (missing antenv.axon_hooks shim)."""
    import types
    import ctypes
    import contextlib

    try:
        import antenv.axon_hooks  # noqa: F401

        return
    except ImportError:
        pass

    so_path = "/opt/axon/libaxon_pjrt.so"
    try:
        lib = ctypes.CDLL(so_path)
    except OSError:
        return
    if not hasattr(lib, "axon_start_nrt_profile"):
        return
    lib.axon_start_nrt_profile.argtypes = [
        ctypes.POINTER(ctypes.c_int64),
        ctypes.c_size_t,
    ]
    lib.axon_start_nrt_profile.restype = ctypes.c_int64
    lib.axon_stop_nrt_profile.argtypes = [ctypes.c_char_p]
    lib.axon_stop_nrt_profile.restype = ctypes.c_int64

    @contextlib.contextmanager
    def _hook(output_dir, device_ids):
        import jax

        jax.devices()
        if device_ids:
            ids = (ctypes.c_int64 * len(device_ids))(*device_ids)
            rc = lib.axon_start_nrt_profile(ids, len(device_ids))
        else:
            rc = lib.axon_start_nrt_profile(None, 0)
        if rc != 0:
            raise RuntimeError(f"axon_start_nrt_profile rc={rc}")
        try:
            yield
        finally:
            n = lib.axon_stop_nrt_profile(str(output_dir).encode())
            print(f"profile: {n} file(s) written to {output_dir}")

    import antenv

    mod = types.ModuleType("antenv.axon_hooks")
    _state = {"hook": _hook}
    mod.get_axon_ntff_profile_hook = lambda: _state["hook"]
    mod.set_axon_ntff_profile_hook = lambda h: _state.__setitem__("hook", h)
    sys.modules["antenv.axon_hooks"] = mod
    antenv.axon_hooks = mod

    # artifact upload has no bucket access in this container
    from concourse import bass_utils

    bass_utils.upload_artifacts = lambda tmpdir: f"local:{tmpdir}"


def _pad_h(a, axis):
    pad = [(0, 0)] * a.ndim
    pad[axis] = (0, HP - a.shape[axis])
    return np.pad(a, pad)


def kernel(
    embed,
    linear_w,
    linear_b,
    start_trans,
    trans,
    end_trans,
    label_emb,
    U_w,
    U_b,
    W_w,
    W_b,
    V_w,
    none_idx,
):
    import ml_dtypes

    bf = ml_dtypes.bfloat16

    embed = np.asarray(embed, np.float32)
    linear_w = np.asarray(linear_w, np.float32)
    linear_b = np.asarray(linear_b, np.float32)
    start_trans = np.asarray(start_trans, np.float32)
    trans = np.asarray(trans, np.float32)
    end_trans = np.asarray(end_trans, np.float32)
    label_emb = np.asarray(label_emb, np.float32)
    U_w = np.asarray(U_w, np.float32)
    U_b = np.asarray(U_b, np.float32)
    W_w = np.asarray(W_w, np.float32)
    W_b = np.asarray(W_b, np.float32)
    V_w = np.asarray(V_w, np.float32)

    # --- host: NER head + Viterbi + hx assembly ---
    ner = embed @ linear_w + linear_b                        # [B,S,T]
    tags = _viterbi_np(ner, start_trans, trans, end_trans)   # [B,S] int32
    hx = np.concatenate([embed, label_emb[tags]], axis=-1)   # [B,S,H]

    # --- device input shards ---
    # pre-transposed to the on-chip [partition, chunk, free] layout so each
    # tensor is one contiguous DMA
    hxTp = _pad_h(hx.transpose(0, 2, 1), 1).astype(bf)       # [B, HP, S]
    uwP = np.ascontiguousarray(
        _pad_h(_pad_h(U_w, 0), 1).reshape(NCH, 128, HP).transpose(1, 0, 2).astype(bf)
    )
    wwP = np.ascontiguousarray(
        _pad_h(_pad_h(W_w, 0), 1).reshape(NCH, 128, HP).transpose(1, 0, 2).astype(bf)
    )
    vP = np.ascontiguousarray(
        _pad_h(V_w, 0).reshape(NCH, 128, R).transpose(1, 0, 2).astype(bf)
    )
    ubwbP = np.ascontiguousarray(
        _pad_h(U_b + W_b, 0).reshape(NCH, 128).T.astype(np.float32)
    )

    in_maps = []
    for c in range(NCORES):
        b, ih = c // 2, c % 2
        in_maps.append(
            {
                "hxT": np.ascontiguousarray(
                    hxTp[b].reshape(NCH, 128, 256).transpose(1, 0, 2)
                ),
                "hxiT": np.ascontiguousarray(
                    hxTp[b][:, ih * 128 : (ih + 1) * 128]
                    .reshape(NCH, 128, 128)
                    .transpose(1, 0, 2)
                ),
                "uw": uwP,
                "ww": wwP,
                "vw": vP,
                "ubwb": ubwbP,
            }
        )

    if "nc" not in _CACHE:
        _CACHE["nc"] = _build()
    nc = _CACHE["nc"]

    from concourse import bass_utils

    trace = bool(int(os.environ.get("BK_TRACE", "0")))
    kw = {}
    if trace:
        _ensure_ntff_hook()
        tdir = os.environ.get("BK_TRACE_DIR")
        if tdir:
            os.makedirs(tdir, exist_ok=True)
            kw["tmpdir"] = tdir
    res = bass_utils.run_bass_kernel_spmd(
        nc, in_maps, core_ids=list(range(NCORES)), trace=trace, **kw
    )
    _CACHE["last_exec_time_ns"] = res.exec_time_ns

    # --- gather/unshard ---
    # out[k][r, j*2+iL] is scores for target i = 2k+iL, source j, relation r
    scores = np.empty((B, S, S, R), np.float32)
    for c in range(NCORES):
        b, ih = c // 2, c % 2
        o = np.asarray(res.results[c]["out"])                # [64, R, 512]
        o = o.reshape(64, R, 256, 2).transpose(0, 3, 2, 1).reshape(128, 256, R)
        scores[b, ih * 128 : (ih + 1) * 128] = o

    ii = np.arange(S)
    scores[:, ii, ii, int(none_idx)] = 100.0
    return tags, scores


# revision 37
# speedup vs baseline: 1.0222x; 1.0222x over previous
"""Trainium2 kernel for joint NER (linear + CRF Viterbi) + biaffine relation scoring.

Strategy:
  - Host (numpy): tiny NER linear [4,256,768]@[768,9], sequential Viterbi DP
    (255 steps over a [4,9] state -- latency-bound, ~0.1% of FLOPs), label
    embedding gather, final diagonal set.
  - Device (8 NeuronCores, Bass/Tile): the heavy biaffine part.
    Shard over (batch b, target-half ih): core c -> b=c//2, i in [ih*128,(ih+1)*128).
    Per core (h padded 798 -> 896 = 7*128 chunks; pad rows hit V=0 -> no effect):
      uT[h,i] = (U_w.T @ hx_b.T)[h, i-slice] + (U_b+W_b)[h]  (bf16 mm, fp32 psum)
      wT[h,j] = (W_w.T @ hx_b.T)[h,j], duplicated 2x -> wsb2[h, j, 2]
      pair[h, ip, j, 2] = tanh(wsb2 + u-broadcast)     (DVE 2x_1p bf16 add, ACT tanh)
      out[i, r, (j,i2)] = sum_h V[h,r] * pair          (PE bf16, psum fp32)
    The 2-wide i2 inner axis keeps every DVE operand packed (step-1 innermost,
    2-byte -> 2x_1p) while the matmul moving operand stays contiguous. The
    mostly-pad 7th h-chunk is packed 4x across partitions so its tanh costs
    512 free cycles instead of 2048, then DMA-unpacked for the matmul.
    Pipeline: 17 i-blocks, pair bufs=4; warm-up matmuls ramp the PE clock
    during the weight DMAs; block-0 adds stream inline behind phase B.
"""

import sys
import os
import numpy as np

sys.path.insert(0, "/opt/trn_rl_repo")

B, S, D, T, L, R = 4, 256, 768, 9, 30, 12
H = D + L          # 798
HP = 896           # 7 * 128, padded h
NCH = 7            # h chunks of 128
IB = 16            # i-values per block
NBLK = 128 // IB   # 8 blocks per core
NCORES = 8

_CACHE = {}


def _viterbi_np(emissions, start_trans, trans, end_trans):
    """Batched Viterbi decode, numpy, matches the jax reference step-for-step."""
    b, s, t = emissions.shape
    score = start_trans[None, :] + emissions[:, 0]            # [B,T]
    hist = np.zeros((s - 1, b, t), np.int64)
    for step in range(1, s):
        nxt = score[:, :, None] + trans[None, :, :]           # [B,Tprev,Tnext]
        hist[step - 1] = np.argmax(nxt, axis=1)
        score = np.max(nxt, axis=1) + emissions[:, step]
    last = np.argmax(score + end_trans[None, :], axis=1)      # [B]
    tags = np.zeros((b, s), np.int32)
    tags[:, -1] = last
    cur = last
    ar = np.arange(b)
    for step in range(s - 2, -1, -1):
        cur = hist[step][ar, cur]
        tags[:, step] = cur
    return tags


def _build():
    """Build + compile the SPMD Bass program (one program, 8 data shards)."""
    from concourse import bass, mybir, tile, bacc

    f32 = mybir.dt.float32
    bf16 = mybir.dt.bfloat16
    ADD = mybir.AluOpType.add
    Tanh = mybir.ActivationFunctionType.Tanh

    nc = bacc.Bacc("TRN2", target_bir_lowering=False, debug=False, num_devices=NCORES)

    hxT_d = nc.dram_tensor("hxT", (128, NCH, 256), bf16, kind="ExternalInput").ap()
    hxiT_d = nc.dram_tensor("hxiT", (128, NCH, 128), bf16, kind="ExternalInput").ap()
    uw_d = nc.dram_tensor("uw", (128, NCH, HP), bf16, kind="ExternalInput").ap()
    ww_d = nc.dram_tensor("ww", (128, NCH, HP), bf16, kind="ExternalInput").ap()
    vw_d = nc.dram_tensor("vw", (128, NCH, R), bf16, kind="ExternalInput").ap()
    ubwb_d = nc.dram_tensor("ubwb", (128, NCH), f32, kind="ExternalInput").ap()
    out_d = nc.dram_tensor("out", (64, R, 512), f32, kind="ExternalOutput").ap()

    with tile.TileContext(nc) as tc:
        with tc.tile_pool(name="persist", bufs=1) as pers:
            v_sb = pers.tile([128, NCH, R], bf16, tag="v")
            ubwb_sb = pers.tile([128, NCH], f32, tag="ubwb")
            # w replicated 2x along an inner i-pair axis: keeps every DVE
            # operand packed (2x_1p) while the matmul rhs stays contiguous.
            # One tile per chunk so phase-D deps are per-chunk exact.
            wsb2 = [
                pers.tile([128, 256, 2], bf16, tag=f"wsb2_{hc}", name=f"wsb2_{hc}")
                for hc in range(NCH)
            ]
            usb = pers.tile([128, NCH, 128], bf16, tag="usb")

            warm_sb = pers.tile([1, 1], f32, tag="warm")
            nc.sync.dma_start(ubwb_sb[:], ubwb_d[:])
            nc.sync.dma_start(v_sb[:], vw_d[:])
            # preload the Tanh ACT table off the critical path
            nc.scalar.activation(warm_sb[:], ubwb_sb[0:1, 0:1], Tanh)

            with (
                tc.tile_pool(name="wts", bufs=1) as wts,
                tc.tile_pool(name="bpsum", bufs=2, space="PSUM") as bp,
                tc.tile_pool(name="pair", bufs=4) as pp,
                tc.tile_pool(name="ostage", bufs=3) as op_,
                tc.tile_pool(name="spsum", bufs=6, space="PSUM") as sp,
            ):
                hxiT_sb = wts.tile([128, NCH, 128], bf16, tag="hxiT")
                uw_sb = wts.tile([128, NCH, HP], bf16, tag="uw")
                hxT_sb = wts.tile([128, NCH, 256], bf16, tag="hxT")
                ww_sb = wts.tile([128, NCH, HP], bf16, tag="ww")
                nc.sync.dma_start(hxiT_sb[:], hxiT_d[:])
                nc.sync.dma_start(uw_sb[:], uw_d[:])
                nc.sync.dma_start(hxT_sb[:], hxT_d[:])
                nc.sync.dma_start(ww_sb[:], ww_d[:])

                # keep the PE busy while the weight DMAs land so the HAM
                # clock-gate ramps to 2.4GHz before the real matmuls start
                wk = wts.tile([128, 512], bf16, tag="wk")
                nc.gpsimd.memset(wk[:], 0)
                wkp = bp.tile([128, 512], f32, tag="bp", name="warmps")
                for r_ in range(40):
                    nc.tensor.matmul(
                        wkp[:], wk[:, 0:128], wk[:], start=True, stop=True
                    )

                # uT first; fold (U_b + W_b) into usb so the wT copy can be
                # a plain ACT-engine Copy (ACT is idle during the prefix)
                for hc in range(NCH):
                    up = bp.tile([128, 128], f32, tag="bp", name=f"up{hc}")
                    for kc in range(NCH):
                        nc.tensor.matmul(
                            up[:],
                            uw_sb[:, kc, hc * 128 : (hc + 1) * 128],
                            hxiT_sb[:, kc, :],
                            start=(kc == 0),
                            stop=(kc == NCH - 1),
                        )
                    nc.vector.tensor_scalar(
                        usb[:, hc, :], up[:], ubwb_sb[:, hc : hc + 1], None, ADD
                    )
                pt0 = pp.tile([128, NCH, 4, 256, 2], bf16, tag="pair", name="pt0")
                for hc in range(NCH):
                    wp = bp.tile([128, 256], f32, tag="bp", name=f"wp{hc}")
                    for kc in range(NCH):
                        nc.tensor.matmul(
                            wp[:],
                            ww_sb[:, kc, hc * 128 : (hc + 1) * 128],
                            hxT_sb[:, kc, :],
                            start=(kc == 0),
                            stop=(kc == NCH - 1),
                        )
                    # one broadcast-read copy fills both duplicate slots
                    nc.scalar.copy(
                        wsb2[hc][:], wp[:].unsqueeze(2).to_broadcast((128, 256, 2))
                    )
                    if hc < 6:
                        # block 0's add immediately behind each chunk, so the
                        # first tanh isn't gated on the whole wT stream
                        u_v0 = (
                            usb[:, hc, 0:8]
                            .rearrange("p (a b) -> p a b", a=4)
                            .unsqueeze(2)
                            .to_broadcast((128, 4, 256, 2))
                        )
                        w_b0 = (
                            wsb2[hc][:]
                            .unsqueeze(1)
                            .to_broadcast((128, 4, 256, 2))
                        )
                        nc.vector.tensor_tensor(pt0[:, hc], w_b0, u_v0, ADD)

                # chunk 6 holds only 30 real h rows; pack its w/u 4x along
                # partitions so its tanh costs 512 free cycles instead of 2048
                u6p = pers.tile([120, 128], bf16, tag="u6p")
                w6p = pers.tile([120, 64, 2], bf16, tag="w6p")
                for q in range(4):
                    nc.sync.dma_start(u6p[q * 30 : (q + 1) * 30, :], usb[0:30, 6, :])
                    nc.sync.dma_start(
                        w6p[q * 30 : (q + 1) * 30],
                        wsb2[6][0:30, q * 64 : (q + 1) * 64, :],
                    )

                # 15 blocks of 8 i, then 2 of 4 i (smaller tail after the
                # last tanh)
                blocks = [(hb * 8, 8) for hb in range(15)] + [(120, 4), (124, 4)]
                pending = []  # (pss, i0, a) whose psum->sbuf drain is deferred
                for i0, cnt in blocks:
                    a = cnt // 2  # i-pairs in this block
                    # pair layout [h, (hc, ipair, j, i2)]: i2 innermost keeps
                    # DVE packed; (j, i2) is contiguous 512 for the matmul rhs
                    if i0 == 0:
                        pt = pt0  # adds already emitted inline with phase B
                    else:
                        pt = pp.tile([128, NCH, a, 256, 2], bf16, tag="pair")
                        for hc in range(6):
                            u_v = (
                                usb[:, hc, i0 : i0 + cnt]
                                .rearrange("p (a b) -> p a b", a=a)
                                .unsqueeze(2)
                                .to_broadcast((128, a, 256, 2))
                            )
                            w_b = (
                                wsb2[hc][:]
                                .unsqueeze(1)
                                .to_broadcast((128, a, 256, 2))
                            )
                            nc.vector.tensor_tensor(pt[:, hc], w_b, u_v, ADD)
                    # chunk 6 in the packed [120, ...] layout
                    p6 = pp.tile([120, a, 64, 2], bf16, tag="p6")
                    u6v = (
                        u6p[:, i0 : i0 + cnt]
                        .rearrange("p (a b) -> p a b", a=a)
                        .unsqueeze(2)
                        .to_broadcast((120, a, 64, 2))
                    )
                    w6v = w6p[:].unsqueeze(1).to_broadcast((120, a, 64, 2))
                    nc.vector.tensor_tensor(p6[:], w6v, u6v, ADD)
                    nc.scalar.activation(
                        p6[:].rearrange("p a c d -> p (a c d)"),
                        p6[:].rearrange("p a c d -> p (a c d)"),
                        Tanh,
                    )
                    # unpack the 4 j-quarters back to [30, ...] for the matmul
                    for q in range(4):
                        nc.sync.dma_start(
                            pt[0:30, 6, :, q * 64 : (q + 1) * 64, :],
                            p6[q * 30 : (q + 1) * 30],
                        )
                    nc.scalar.activation(
                        pt[:, 0:6].rearrange("p a b c d -> p (a b c d)"),
                        pt[:, 0:6].rearrange("p a b c d -> p (a b c d)"),
                        Tanh,
                    )

                    # hc-outer: 4 psum tiles alive, 4 back-to-back matmuls
                    # per stationary V chunk load
                    pss = [
                        sp.tile([128, 512], f32, tag="ps", name=f"ps_{i0}_{t}")
                        for t in range(a)
                    ]
                    for hc in range(6):
                        for t in range(a):
                            nc.tensor.matmul(
                                pss[t][0:R, :],
                                v_sb[:, hc, :],
                                pt[:, hc, t],
                                start=(hc == 0),
                                stop=False,
                            )
                    for t in range(a):
                        # chunk 6: contract only the 30 real h rows
                        nc.tensor.matmul(
                            pss[t][0:R, :],
                            v_sb[0:30, 6, :],
                            pt[0:30, 6, t],
                            start=False,
                            stop=True,
                        )
                    # drain half now, defer half until after the next
                    # block's adds -- keeps DVE feeding ACT first while only
                    # 6 psum banks stay live
                    for t in range(2, a):
                        osb = op_.tile([128, 512], f32, tag="osb")
                        nc.vector.tensor_copy(osb[0:R, :], pss[t][0:R, :])
                        nc.sync.dma_start(out_d[i0 // 2 + t], osb[0:R, :])
                    pending.append((pss, i0, min(a, 2)))
                    if len(pending) > 1:
                        ppss, pi0, pa = pending.pop(0)
                        for t in range(pa):
                            osb = op_.tile([128, 512], f32, tag="osb")
                            nc.vector.tensor_copy(osb[0:R, :], ppss[t][0:R, :])
                            nc.sync.dma_start(out_d[pi0 // 2 + t], osb[0:R, :])
                for ppss, pi0, pa in pending:
                    for t in range(pa):
                        osb = op_.tile([128, 512], f32, tag="osb")
                        nc.vector.tensor_copy(osb[0:R, :], ppss[t][0:R, :])
                        nc.sync.dma_start(out_d[pi0 // 2 + t], osb[0:R, :])

    nc.compile()
    return nc


def _ensure_ntff_hook():
    """Register the axon NTFF profiling hook (missing antenv.axon_hooks shim)."""
    import types
    import ctypes
    import contextlib

    try:
        import antenv.axon_hooks  # noqa: F401

        return
    except ImportError:
        pass

    so_path = "/opt/axon/libaxon_pjrt.so"
    try:
        lib = ctypes.CDLL(so_path)
    except OSError:
        return
    if not hasattr(lib, "axon_start_nrt_profile"):
        return
    lib.axon_start_nrt_profile.argtypes = [
        ctypes.POINTER(ctypes.c_int64),
        ctypes.c_size_t,
    ]
    lib.axon_start_nrt_profile.restype = ctypes.c_int64
    lib.axon_stop_nrt_profile.argtypes = [ctypes.c_char_p]
    lib.axon_stop_nrt_profile.restype = ctypes.c_int64

    @contextlib.contextmanager
    def _hook(output_dir, device_ids):
        import jax

        jax.devices()
        if device_ids:
            ids = (ctypes.c_int64 * len(device_ids))(*device_ids)
            rc = lib.axon_start_nrt_profile(ids, len(device_ids))
        else:
            rc = lib.axon_start_nrt_profile(None, 0)
        if rc != 0:
            raise RuntimeError(f"axon_start_nrt_profile rc={rc}")
        try:
            yield
        finally:
            n = lib.axon_stop_nrt_profile(str(output_dir).encode())
            print(f"profile: {n} file(s) written to {output_dir}")

    import antenv

    mod = types.ModuleType("antenv.axon_hooks")
    _state = {"hook": _hook}
    mod.get_axon_ntff_profile_hook = lambda: _state["hook"]
    mod.set_axon_ntff_profile_hook = lambda h: _state.__setitem__("hook", h)
    sys.modules["antenv.axon_hooks"] = mod
    antenv.axon_hooks = mod

    # artifact upload has no bucket access in this container
    from concourse import bass_utils

    bass_utils.upload_artifacts = lambda tmpdir: f"local:{tmpdir}"


def _pad_h(a, axis):
    pad = [(0, 0)] * a.ndim
    pad[axis] = (0, HP - a.shape[axis])
    return np.pad(a, pad)


def kernel(
    embed,
    linear_w,
    linear_b,
    start_trans,
    trans,
    end_trans,
    label_emb,
    U_w,
    U_b,
    W_w,
    W_b,
    V_w,
    none_idx,
):
    import ml_dtypes

    bf = ml_dtypes.bfloat16

    embed = np.asarray(embed, np.float32)
    linear_w = np.asarray(linear_w, np.float32)
    linear_b = np.asarray(linear_b, np.float32)
    start_trans = np.asarray(start_trans, np.float32)
    trans = np.asarray(trans, np.float32)
    end_trans = np.asarray(end_trans, np.float32)
    label_emb = np.asarray(label_emb, np.float32)
    U_w = np.asarray(U_w, np.float32)
    U_b = np.asarray(U_b, np.float32)
    W_w = np.asarray(W_w, np.float32)
    W_b = np.asarray(W_b, np.float32)
    V_w = np.asarray(V_w, np.float32)

    # --- host: NER head + Viterbi + hx assembly ---
    ner = embed @ linear_w + linear_b                        # [B,S,T]
    tags = _viterbi_np(ner, start_trans, trans, end_trans)   # [B,S] int32
    hx = np.concatenate([embed, label_emb[tags]], axis=-1)   # [B,S,H]

    # --- device input shards ---
    # pre-transposed to the on-chip [partition, chunk, free] layout so each
    # tensor is one contiguous DMA
    hxTp = _pad_h(hx.transpose(0, 2, 1), 1).astype(bf)       # [B, HP, S]
    uwP = np.ascontiguousarray(
        _pad_h(_pad_h(U_w, 0), 1).reshape(NCH, 128, HP).transpose(1, 0, 2).astype(bf)
    )
    wwP = np.ascontiguousarray(
        _pad_h(_pad_h(W_w, 0), 1).reshape(NCH, 128, HP).transpose(1, 0, 2).astype(bf)
    )
    vP = np.ascontiguousarray(
        _pad_h(V_w, 0).reshape(NCH, 128, R).transpose(1, 0, 2).astype(bf)
    )
    ubwbP = np.ascontiguousarray(
        _pad_h(U_b + W_b, 0).reshape(NCH, 128).T.astype(np.float32)
    )

    in_maps = []
    for c in range(NCORES):
        b, ih = c // 2, c % 2
        in_maps.append(
            {
                "hxT": np.ascontiguousarray(
                    hxTp[b].reshape(NCH, 128, 256).transpose(1, 0, 2)
                ),
                "hxiT": np.ascontiguousarray(
                    hxTp[b][:, ih * 128 : (ih + 1) * 128]
                    .reshape(NCH, 128, 128)
                    .transpose(1, 0, 2)
                ),
                "uw": uwP,
                "ww": wwP,
                "vw": vP,
                "ubwb": ubwbP,
            }
        )

    if "nc" not in _CACHE:
        _CACHE["nc"] = _build()
    nc = _CACHE["nc"]

    from concourse import bass_utils

    trace = bool(int(os.environ.get("BK_TRACE", "0")))
    kw = {}
    if trace:
        _ensure_ntff_hook()
        tdir = os.environ.get("BK_TRACE_DIR")
        if tdir:
            os.makedirs(tdir, exist_ok=True)
            kw["tmpdir"] = tdir
    res = bass_utils.run_bass_kernel_spmd(
        nc, in_maps, core_ids=list(range(NCORES)), trace=trace, **kw
    )
    _CACHE["last_exec_time_ns"] = res.exec_time_ns

    # --- gather/unshard ---
    # out[k][r, j*2+iL] is scores for target i = 2k+iL, source j, relation r
    scores = np.empty((B, S, S, R), np.float32)
    for c in range(NCORES):
        b, ih = c // 2, c % 2
        o = np.asarray(res.results[c]["out"])                # [64, R, 512]
        o = o.reshape(64, R, 256, 2).transpose(0, 3, 2, 1).reshape(128, 256, R)
        scores[b, ih * 128 : (ih + 1) * 128] = o

    ii = np.arange(S)
    scores[:, ii, ii, int(none_idx)] = 100.0
    return tags, scores
